# revision 1
# baseline (speedup 1.0000x reference)
"""EmbeddingBag(mean, 1M x 128 table) + Linear(128->5) on 8 Trainium2 cores.

Strategy (data-parallel by bags, table replicated per core, bf16 gather):
  - Each core owns 2048 consecutive bags (a contiguous slice of
    sparse_features since offsets are sorted), ~102K indices/core.
  - The 1M-row table is split into 32 windows of 31250 rows so row offsets
    fit the int16 indices of the batched `dma_gather` custom instruction
    (one instruction gathers a whole cell = all of a block's positions that
    hit one window; 4 SWDGE queues run descriptor generation in parallel).
  - Bags are grouped into 8 blocks of 256 slots; each block's positions are
    sorted by window, cell lengths padded to 128 and shared across cores
    (SPMD uniformity), ~20% padding.
  - Pooling: gathered 128-position tiles [pos->partition, dim->free] are
    multiplied on TensorE by an on-chip 0/1 selection matrix S
    (is_equal(slot_id, iota), built alternately on DVE and ACT), accumulating
    bag sums for all 2048 slots directly in PSUM ([dim, slot] layout).
  - Mean = multiply by precomputed 1/count, Linear = 4 fp32 matmuls
    contracting over dim, bias added on DVE. No collectives needed.
  - Table/gather/S run in bf16 (~1.7e-3 rel err vs fp32 reference);
    accumulation and the Linear stay fp32.
"""
import sys

if '/opt/trn_rl_repo' not in sys.path:
    sys.path.insert(0, '/opt/trn_rl_repo')

import numpy as np
import ml_dtypes

# Problem constants (nn_Net_2 embedding_lookup).
NUM_EMB = 1_000_000
D = 128
BATCH = 16384
OUT_DIM = 5
NCORES = 8
BPC = BATCH // NCORES       # bags per core
SLOT = 256                  # bag slots per block (matmul rhs width)
NBLK = BPC // SLOT          # blocks per core (8)
WIN = 31250                 # table rows per int16 window
NWIN = NUM_EMB // WIN       # 32
S_CH = 32                   # tiles per S-build chunk
MAX_CELL_TILES = 8          # split bigger cells into multiple gathers
NQUEUES = 4


def build_plan(sparse_features, offsets):
    """Shard bags over cores; per (core, block) sort positions by table
    window; pad every (block, window) cell to a 128-multiple length shared
    across cores so the SPMD program is uniform."""
    sf = np.asarray(sparse_features).astype(np.int64)
    offsets = np.asarray(offsets).astype(np.int64)
    counts = np.diff(offsets)

    # positions per (core, block, window) + their idx/slot lists
    cell_items = {}   # (c, j, w) -> [idx_local array, slot array]
    for c in range(NCORES):
        for j in range(NBLK):
            bag0 = c * BPC + j * SLOT
            lo, hi = offsets[bag0], offsets[bag0 + SLOT]
            idxs = sf[lo:hi]
            # slot id per position within this block
            reps = counts[bag0:bag0 + SLOT]
            slots = np.repeat(np.arange(SLOT), reps)
            w = idxs // WIN
            order = np.argsort(w, kind="stable")
            idxs, slots, w = idxs[order], slots[order], w[order]
            bnd = np.searchsorted(w, np.arange(NWIN + 1))
            for win in range(NWIN):
                a, b = bnd[win], bnd[win + 1]
                if b > a:
                    cell_items[(c, j, win)] = (idxs[a:b] - win * WIN, slots[a:b])

    # shared (across cores) padded cell lengths
    cell_len = np.zeros((NBLK, NWIN), dtype=np.int64)
    for j in range(NBLK):
        for w in range(NWIN):
            m = max(
                (len(cell_items[(c, j, w)][0]) if (c, j, w) in cell_items else 0)
                for c in range(NCORES)
            )
            cell_len[j, w] = -(-m // 128) * 128  # ceil to 128

    # layout: blocks in order, cells in window order within block
    gathers = []      # (tile_off, ntiles, window, idx_col_off) shared by cores
    blk_tile0 = []    # first tile of each block
    t_off = 0
    col_off = 0
    for j in range(NBLK):
        blk_tile0.append(t_off)
        for w in range(NWIN):
            L = int(cell_len[j, w])
            if L == 0:
                continue
            # split cells bigger than MAX_CELL_TILES tiles
            done = 0
            while done < L:
                piece = min(L - done, MAX_CELL_TILES * 128)
                gathers.append((t_off + done // 128, piece // 128, w, col_off))
                done += piece
                col_off += piece // 16
            t_off += L // 128
    blk_tile0.append(t_off)
    nt = t_off
    ncols = col_off

    cores = []
    for c in range(NCORES):
        idx16 = np.zeros((128, ncols), dtype=np.int16)
        slot_id = np.full((NBLK, NWIN and 0 or 0,), 0)  # placeholder
        slot_flat = np.full(nt * 128, SLOT, dtype=np.float32)
        recip = np.ones(BPC, dtype=np.float32)
        cnt = counts[c * BPC:(c + 1) * BPC]
        recip[:] = 1.0 / np.maximum(cnt, 1)
        t_off = 0
        col = 0
        for j in range(NBLK):
            for w in range(NWIN):
                L = int(cell_len[j, w])
                if L == 0:
                    continue
                item = cell_items.get((c, j, w))
                cell_idx = np.zeros(L, dtype=np.int16)
                cell_slot = np.full(L, SLOT, dtype=np.float32)
                if item is not None:
                    n = len(item[0])
                    cell_idx[:n] = item[0]
                    cell_slot[:n] = item[1]
                base = t_off * 128
                slot_flat[base:base + L] = cell_slot
                # idx16 wrapped-in-16, replicated over the 8 core groups
                wrapped = cell_idx.reshape(L // 16, 16).T     # [16, L/16]
                for g8 in range(8):
                    idx16[g8 * 16:(g8 + 1) * 16, col:col + L // 16] = wrapped
                t_off += L // 128
                col += L // 16
        # tile layout: position i -> partition i%128, tile i//128
        slot_t = slot_flat.reshape(nt, 128).T                  # [128, nt]
        cores.append({
            "idx16": idx16,
            "slot": np.ascontiguousarray(slot_t.astype(ml_dtypes.bfloat16)),
            "recipb": np.ascontiguousarray(
                np.broadcast_to(recip, (128, BPC)).astype(np.float32)
            ),
        })
    return {
        "nt": nt,
        "ncols": ncols,
        "gathers": gathers,
        "blk_tile0": blk_tile0,
        "cores": cores,
    }


def simulate_plan(plan, emb_table, lin_w, lin_b):
    """Numpy emulation of the device computation (bf16 table/S)."""
    emb = np.asarray(emb_table).astype(ml_dtypes.bfloat16).astype(np.float32)
    out = np.zeros((BATCH, OUT_DIM), dtype=np.float32)
    nt = plan["nt"]
    for c in range(NCORES):
        pc = plan["cores"][c]
        slot_t = pc["slot"].astype(np.float32)
        pooled = np.zeros((D, BPC), dtype=np.float32)
        # reconstruct gathered rows from idx16 + gather list
        for (t0, ntl, w, col) in plan["gathers"]:
            wrapped = pc["idx16"][:16, col:col + ntl * 8]     # [16, L/16]
            cell_idx = wrapped.T.reshape(-1).astype(np.int64) + w * WIN
            g = emb[cell_idx].reshape(ntl, 128, D)            # [ntl, 128, D]
            j = np.searchsorted(plan["blk_tile0"], t0, side="right") - 1
            for u in range(ntl):
                t = t0 + u
                s = (slot_t[:, t:t + 1] == np.arange(SLOT)[None, :]).astype(np.float32)
                pooled[:, j * SLOT:(j + 1) * SLOT] += g[u].T @ s
        pooled *= pc["recipb"]
        lin = np.asarray(lin_w) @ pooled + np.asarray(lin_b)[:, None]
        out[c * BPC:(c + 1) * BPC] = lin.T
    return out


def build_program(plan):
    from concourse import bacc, mybir
    import concourse.tile as tile

    f32 = mybir.dt.float32
    bf16 = mybir.dt.bfloat16
    i16 = mybir.dt.int16
    nt, ncols = plan["nt"], plan["ncols"]
    gathers, blk_tile0 = plan["gathers"], plan["blk_tile0"]

    nc = bacc.Bacc("TRN2", debug=False, num_swdge_queues=NQUEUES)
    emb_d = nc.declare_dram_parameter("emb", [NUM_EMB, D], bf16, isOutput=False)
    idx_d = nc.declare_dram_parameter("idx", [128, ncols], i16, isOutput=False)
    slt_d = nc.declare_dram_parameter("slt", [128, nt], bf16, isOutput=False)
    iot_d = nc.declare_dram_parameter("iot", [128, S_CH, SLOT], bf16, isOutput=False)
    rcp_d = nc.declare_dram_parameter("rcp", [128, BPC], f32, isOutput=False)
    wt_d = nc.declare_dram_parameter("wt", [128, OUT_DIM], f32, isOutput=False)
    bia_d = nc.declare_dram_parameter("bia", [128, 1], f32, isOutput=False)
    out_d = nc.declare_dram_parameter("out", [OUT_DIM, BPC], f32, isOutput=True)

    # tile index -> block
    t2j = np.searchsorted(blk_tile0, np.arange(nt), side="right") - 1

    with tile.TileContext(nc) as tc:
        with (
            tc.tile_pool(name="const", bufs=1) as const_p,
            tc.tile_pool(name="gbuf", bufs=12) as g_p,
            tc.tile_pool(name="sbuf", bufs=4) as s_p,
            tc.tile_pool(name="res", bufs=1) as res_p,
            tc.tile_pool(name="psum", bufs=1, space="PSUM") as psum_p,
        ):
            idx_sb = const_p.tile([128, ncols], i16)
            slt_sb = const_p.tile([128, nt], bf16)
            iot_sb = const_p.tile([128, S_CH, SLOT], bf16)
            rcp_sb = const_p.tile([128, BPC], f32)
            wt_sb = const_p.tile([128, OUT_DIM], f32)
            bia_sb = const_p.tile([128, 1], f32)
            nc.sync.dma_start(out=idx_sb[:], in_=idx_d.ap()[:, :])
            nc.sync.dma_start(out=slt_sb[:], in_=slt_d.ap()[:, :])
            nc.sync.dma_start(out=iot_sb[:], in_=iot_d.ap()[:, :, :])
            nc.sync.dma_start(out=rcp_sb[:], in_=rcp_d.ap()[:, :])
            nc.sync.dma_start(out=wt_sb[:], in_=wt_d.ap()[:, :])
            nc.sync.dma_start(out=bia_sb[:], in_=bia_d.ap()[:, :])

            pooled_ps = psum_p.tile([128, BPC], f32)

            # S chunks, alternating DVE / ACT
            s_tiles = {}
            for s0 in range(0, nt, S_CH):
                tiles = min(S_CH, nt - s0)
                s = s_p.tile([128, S_CH, SLOT], bf16, tag="s")
                nc.vector.tensor_tensor(
                    out=s[:, :tiles, :],
                    in0=slt_sb[:, s0:s0 + tiles, None].to_broadcast(
                        [128, tiles, SLOT]
                    ),
                    in1=iot_sb[:, :tiles, :],
                    op=mybir.AluOpType.is_equal,
                )
                s_tiles[s0] = s

            # gather cells + matmuls, in tile order
            qrr = 0
            gather_of_tile = {}
            for (t0, ntl, w, col) in gathers:
                g = g_p.tile([128, MAX_CELL_TILES, D], bf16, tag="g")
                win_n = min(WIN, NUM_EMB - w * WIN)
                nc.gpsimd.dma_gather(
                    out_ap=g[:, :ntl, :],
                    in_ap=emb_d.ap()[w * WIN:w * WIN + win_n, :],
                    idxs_ap=idx_sb[:, col:col + ntl * 8],
                    num_idxs=ntl * 128,
                    num_idxs_reg=ntl * 128,
                    elem_size=D,
                    queue_num=qrr % NQUEUES,
                )
                qrr += 1
                for u in range(ntl):
                    gather_of_tile[t0 + u] = (g, u)

            for t in range(nt):
                j = int(t2j[t])
                g, u = gather_of_tile[t]
                s = s_tiles[(t // S_CH) * S_CH]
                nc.tensor.matmul(
                    out=pooled_ps[:, j * SLOT:(j + 1) * SLOT],
                    lhsT=g[:, u, :],
                    rhs=s[:, t % S_CH, :],
                    start=(t == blk_tile0[j]),
                    stop=(t == blk_tile0[j + 1] - 1),
                )

            pooled_sb = res_p.tile([128, BPC], f32)
            nc.vector.tensor_tensor(
                out=pooled_sb[:],
                in0=pooled_ps[:],
                in1=rcp_sb[:],
                op=mybir.AluOpType.mult,
            )
            out_sb = res_p.tile([OUT_DIM, BPC], f32)
            for k in range(BPC // 512):
                lin_ps = psum_p.tile([128, 512], f32, tag="lin")
                nc.tensor.matmul(
                    out=lin_ps[:OUT_DIM, :],
                    lhsT=wt_sb[:],
                    rhs=pooled_sb[:, k * 512:(k + 1) * 512],
                    start=True,
                    stop=True,
                )
                nc.vector.tensor_tensor(
                    out=out_sb[:, k * 512:(k + 1) * 512],
                    in0=lin_ps[:OUT_DIM, :],
                    in1=bia_sb[:OUT_DIM, 0:1].to_broadcast([OUT_DIM, 512]),
                    op=mybir.AluOpType.add,
                )
            nc.sync.dma_start(out=out_d.ap()[:, :], in_=out_sb[:])

    nc.finalize()
    return nc


def make_in_maps(plan, emb_table, lin_w, lin_b):
    emb_bf = np.ascontiguousarray(
        np.asarray(emb_table, dtype=np.float32).astype(ml_dtypes.bfloat16)
    )
    lin_w = np.asarray(lin_w, dtype=np.float32)
    lin_b = np.asarray(lin_b, dtype=np.float32)
    iota = np.broadcast_to(
        np.arange(SLOT, dtype=np.float32).astype(ml_dtypes.bfloat16),
        (128, S_CH, SLOT),
    ).copy()
    wt = np.ascontiguousarray(lin_w.T)
    bia = np.zeros((128, 1), dtype=np.float32)
    bia[:OUT_DIM, 0] = lin_b
    in_maps = []
    for c in range(NCORES):
        pc = plan["cores"][c]
        in_maps.append({
            "emb": emb_bf,
            "idx": pc["idx16"],
            "slt": pc["slot"],
            "iot": iota,
            "rcp": pc["recipb"],
            "wt": wt,
            "bia": bia,
        })
    return in_maps


def assemble_output(results):
    out = np.zeros((BATCH, OUT_DIM), dtype=np.float32)
    for c in range(NCORES):
        out[c * BPC:(c + 1) * BPC] = results[c]["out"].T
    return out


def kernel(emb_table, lin_w, lin_b, sparse_features, offsets, send_shape,
           trace=False):
    from concourse.bass_utils import run_bass_kernel_spmd

    plan = build_plan(sparse_features, offsets)
    nc = build_program(plan)
    in_maps = make_in_maps(plan, emb_table, lin_w, lin_b)
    res = run_bass_kernel_spmd(nc, in_maps, list(range(NCORES)), trace=trace)
    out = assemble_output(res.results)
    if trace:
        return out, res
    return out



# revision 18
# speedup vs baseline: 1.7808x; 1.7808x over previous
"""EmbeddingBag(mean, 1M x 128 table) + Linear(128->5) on 8 Trainium2 cores.

Strategy (data-parallel by bags, table replicated per core, bf16 gather):
  - Each core owns 2048 consecutive bags (a contiguous slice of
    sparse_features since offsets are sorted), ~102K indices/core.
  - The 1M-row table is split into 32 windows of 31250 rows so row offsets
    fit the int16 indices of the batched `dma_gather` custom instruction.
    ONE gather instruction per window (32 per core, vs 256 in the naive
    cell-based layout) keeps the serial SWDGE descriptor-generation cost
    on GpSimd small (~994 ns fixed per instruction).
  - Positions are sorted by (window, bag). Window lengths are padded to a
    shared ceil-128 max across cores (~5%), with each core's pads
    interleaved at matched quantiles so the slot progression at any tile
    rank is nearly identical across cores.
  - Pooling: gathered 128-position tiles [pos->partition, dim->free] are
    multiplied on TensorE by an on-chip 0/1 selection matrix S built with
    is_equal against an iota, accumulating bag sums in PSUM [dim, slot].
    Because positions are slot-sorted, each tile only spans a narrow slot
    range: the matmul writes a per-tile sliding PSUM window
    [B(t), B(t)+W(t)) (W ~ 135 avg) instead of a fixed 256-wide block,
    cutting both TensorE columns and the S-build element count ~2x.
    PSUM is pre-zeroed; all pooling matmuls accumulate (start=False).
    Matmuls are split at PSUM bank (512-col) boundaries.
  - The S build (is_equal) runs in 16-tile chunks, split between DVE and
    GpSimd (Pool) to balance engine load.
  - Mean = multiply by precomputed 1/count, Linear = 4 fp32 matmuls
    contracting over dim, bias added on DVE. No collectives needed.
  - Table/gather/S run in bf16 (~1.7e-3 rel err vs fp32 reference);
    accumulation and the Linear stay fp32.
"""
import sys

if '/opt/trn_rl_repo' not in sys.path:
    sys.path.insert(0, '/opt/trn_rl_repo')

import numpy as np
import ml_dtypes

# Problem constants (nn_Net_2 embedding_lookup).
NUM_EMB = 1_000_000
D = 128
BATCH = 16384
OUT_DIM = 5
NCORES = 8
BPC = BATCH // NCORES       # bags per core
WIN = 31250                 # table rows per int16 window
NWIN = NUM_EMB // WIN       # 32
K_MAX = 24                  # max tiles per S-build chunk (DP-chosen bounds)
K_FIX = 150                 # per-chunk fixed cost for the chunking DP (cols)
SENTINEL = 384.0            # slotb value for pads (never matches iota)
NQUEUES = 4
G_BUFS = 6                  # gather ring buffers (windows in flight)
S_BUFS = 5                  # S-chunk ring buffers


def build_plan(sparse_features, offsets):
    sf = np.asarray(sparse_features).astype(np.int64)
    offsets = np.asarray(offsets).astype(np.int64)
    counts = np.diff(offsets)
    bag_of = np.repeat(np.arange(BATCH), counts)

    # per-core sorted streams
    core_rows_rel = {}
    core_slots = {}
    L = np.zeros((NCORES, NWIN), dtype=np.int64)
    bnds = {}
    for c in range(NCORES):
        lo, hi = offsets[c * BPC], offsets[(c + 1) * BPC]
        rows = sf[lo:hi]
        slot = bag_of[lo:hi] - c * BPC
        win = rows // WIN
        order = np.lexsort((slot, win))
        rows, slot, win = rows[order], slot[order], win[order]
        bnd = np.searchsorted(win, np.arange(NWIN + 1))
        L[c] = np.diff(bnd)
        core_rows_rel[c] = rows - win * WIN
        core_slots[c] = slot
        bnds[c] = bnd

    P = ((L.max(axis=0) + 127) // 128) * 128      # shared padded window lens
    base_pos = np.concatenate([[0], np.cumsum(P)])  # padded stream offsets
    NPOS = int(P.sum())
    NT = NPOS // 128
    t0 = base_pos // 128                           # first tile of window w
    window_of_tile = np.searchsorted(t0[1:], np.arange(NT), side='right')

    # padded per-core streams with quantile-interleaved pads
    slot_pad = np.full((NCORES, NPOS), -1, dtype=np.int32)
    rows_pad = np.zeros((NCORES, NPOS), dtype=np.int16)
    rows_abs_pad = np.zeros((NCORES, NPOS), dtype=np.int64)  # for simulation
    for c in range(NCORES):
        for w in range(NWIN):
            n = int(L[c, w])
            if n == 0:
                continue
            rk = (np.arange(n) * P[w]) // n + base_pos[w]
            sl = core_slots[c][bnds[c][w]:bnds[c][w + 1]]
            rr = core_rows_rel[c][bnds[c][w]:bnds[c][w + 1]]
            slot_pad[c, rk] = sl
            rows_pad[c, rk] = rr
            rows_abs_pad[c, rk] = rr + w * WIN
        # pads keep idx 0 -> they fetch row w*WIN (harmless; S zeroes them)
        for w in range(NWIN):
            seg = rows_abs_pad[c, base_pos[w]:base_pos[w + 1]]
            seg[slot_pad[c, base_pos[w]:base_pos[w + 1]] < 0] = w * WIN

    # shared per-tile PSUM window [B, B+W)
    sp = slot_pad.reshape(NCORES, NT, 128)
    mask = sp >= 0
    mn = np.where(mask, sp, 1 << 20).min(axis=(0, 2))
    mx = np.where(mask, sp, -1).max(axis=(0, 2))
    W = np.where(mx >= 0, mx - np.where(mn > (1 << 19), 0, mn) + 1, 0)
    B = np.where(W > 0, np.where(mn > (1 << 19), 0, mn), 0)
    assert (B + W).max() <= BPC

    # matmul segments split at PSUM bank (512 fp32 cols) boundaries
    segs = []   # (t, psum_col, s_col, width)
    for t in range(NT):
        if W[t] == 0:
            continue
        b, e = int(B[t]), int(B[t] + W[t])
        c0 = b
        while c0 < e:
            c1 = min(e, (c0 // 512 + 1) * 512)
            segs.append((t, c0, c0 - b, c1 - c0))
            c0 = c1

    # S chunks: DP-chosen boundaries minimizing sum(len * maxW) + fixed cost
    INF = 1 << 60
    cost = np.full(NT + 1, INF, dtype=np.int64)
    cost[0] = 0
    arg = np.zeros(NT + 1, dtype=np.int32)
    for j in range(1, NT + 1):
        wmax = 0
        for i in range(j - 1, max(-1, j - 1 - K_MAX), -1):
            wmax = max(wmax, W[i])
            cc = cost[i] + (j - i) * wmax + K_FIX
            if cc < cost[j]:
                cost[j] = cc
                arg[j] = i
    chunks = []          # (i0, i1, Wc)
    j = NT
    while j > 0:
        i = int(arg[j])
        chunks.append((i, j, int(W[i:j].max())))
        j = i
    chunks.reverse()
    chunk_of_tile = np.zeros(NT, dtype=np.int32)
    for ci, (i0, i1, _) in enumerate(chunks):
        chunk_of_tile[i0:i1] = ci
    K_USED = max(i1 - i0 for (i0, i1, _) in chunks)
    WMAX = int(max(8, ((max(wc for (_, _, wc) in chunks) + 7) // 8) * 8))

    # per-core tensors
    cores = []
    for c in range(NCORES):
        ncols = NPOS // 16
        idx16 = np.zeros((128, ncols), dtype=np.int16)
        for w in range(NWIN):
            seg = rows_pad[c, base_pos[w]:base_pos[w + 1]]
            wrapped = seg.reshape(-1, 16).T      # [16, P/16]
            idx16[:, base_pos[w] // 16: base_pos[w + 1] // 16] = np.tile(
                wrapped, (8, 1)
            )
        slotb = slot_pad[c].astype(np.float32) - B[np.arange(NPOS) // 128]
        slotb[slot_pad[c] < 0] = SENTINEL
        slotb_t = slotb.reshape(NT, 128).T       # [128, NT]
        cnt = counts[c * BPC:(c + 1) * BPC]
        recip = (1.0 / np.maximum(cnt, 1)).astype(np.float32)
        cores.append({
            "idx16": idx16,
            "slotb": np.ascontiguousarray(slotb_t.astype(ml_dtypes.bfloat16)),
            "recipb": np.ascontiguousarray(
                np.broadcast_to(recip, (128, BPC)).astype(np.float32)
            ),
            "rows_abs_pad": rows_abs_pad[c],
            "slot_pad": slot_pad[c],
        })

    return {
        "nt": NT,
        "npos": NPOS,
        "ncols": NPOS // 16,
        "P": P,
        "t0": t0,
        "window_of_tile": window_of_tile,
        "B": B,
        "W": W,
        "segs": segs,
        "chunks": chunks,
        "chunk_of_tile": chunk_of_tile,
        "K_USED": K_USED,
        "WMAX": WMAX,
        "gathers": [(w, int(t0[w]), int(P[w])) for w in range(NWIN) if P[w] > 0],
        "cores": cores,
    }


def simulate_plan(plan, emb_table, lin_w, lin_b):
    """Numpy emulation of the device computation (bf16 table/S)."""
    emb = np.asarray(emb_table).astype(ml_dtypes.bfloat16).astype(np.float32)
    out = np.zeros((BATCH, OUT_DIM), dtype=np.float32)
    NT, B, W = plan["nt"], plan["B"], plan["W"]
    for c in range(NCORES):
        pc = plan["cores"][c]
        g = emb[pc["rows_abs_pad"]].reshape(NT, 128, D)
        slotb = pc["slotb"].astype(np.float32)   # [128, NT]
        pooled = np.zeros((D, BPC), dtype=np.float32)
        for t in range(NT):
            if W[t] == 0:
                continue
            s = (slotb[:, t:t + 1] == np.arange(W[t])[None, :]).astype(np.float32)
            pooled[:, B[t]:B[t] + W[t]] += g[t].T @ s
        pooled *= pc["recipb"]
        lin = np.asarray(lin_w) @ pooled + np.asarray(lin_b)[:, None]
        out[c * BPC:(c + 1) * BPC] = lin.T
    return out


def build_program(plan):
    from concourse import bacc, mybir
    import concourse.tile as tile

    f32 = mybir.dt.float32
    bf16 = mybir.dt.bfloat16
    i16 = mybir.dt.int16
    NT, ncols = plan["nt"], plan["ncols"]
    P, t0 = plan["P"], plan["t0"]
    W, segs = plan["W"], plan["segs"]
    chunks, chunk_of_tile = plan["chunks"], plan["chunk_of_tile"]
    K_USED, WMAX = plan["K_USED"], plan["WMAX"]
    window_of_tile = plan["window_of_tile"]
    NT_W = int(P.max()) // 128

    nc = bacc.Bacc("TRN2", debug=False, num_swdge_queues=NQUEUES)
    emb_d = nc.declare_dram_parameter("emb", [NUM_EMB, D], bf16, isOutput=False)
    idx_d = nc.declare_dram_parameter("idx", [128, ncols], i16, isOutput=False)
    slb_d = nc.declare_dram_parameter("slb", [128, NT], bf16, isOutput=False)
    iot_d = nc.declare_dram_parameter("iot", [128, K_USED, WMAX], bf16, isOutput=False)
    rcp_d = nc.declare_dram_parameter("rcp", [128, BPC], f32, isOutput=False)
    wt_d = nc.declare_dram_parameter("wt", [128, OUT_DIM], f32, isOutput=False)
    bia_d = nc.declare_dram_parameter("bia", [128, 1], f32, isOutput=False)
    out_d = nc.declare_dram_parameter("out", [OUT_DIM, BPC], f32, isOutput=True)

    # S chunks (all on DVE; Pool's ISA lacks is_equal) are emitted in
    # ascending order (so the ring-buffer slot allocation order matches the
    # matmuls' consumption order), interleaved into the gather loop right
    # after the gather covering each chunk's last tile.
    chunks_after_gather = {}
    for ci, (i0, i1, wc) in enumerate(chunks):
        chunks_after_gather.setdefault(int(window_of_tile[i1 - 1]), []).append(ci)

    with tile.TileContext(nc) as tc:
        with (
            tc.tile_pool(name="const", bufs=1) as const_p,
            tc.tile_pool(name="gbuf", bufs=G_BUFS) as g_p,
            tc.tile_pool(name="sbuf", bufs=S_BUFS) as s_p,
            tc.tile_pool(name="res", bufs=1) as res_p,
            tc.tile_pool(name="psum", bufs=1, space="PSUM") as psum_p,
            tc.tile_pool(name="psuml", bufs=2, space="PSUM") as psuml_p,
        ):
            idx_sb = const_p.tile([128, ncols], i16)
            slb_sb = const_p.tile([128, NT], bf16)
            iot_sb = const_p.tile([128, K_USED, WMAX], bf16)
            rcp_sb = const_p.tile([128, BPC], f32)
            wt_sb = const_p.tile([128, OUT_DIM], f32)
            bia_sb = const_p.tile([128, 1], f32)
            nc.sync.dma_start(out=idx_sb[:], in_=idx_d.ap()[:, :])
            nc.sync.dma_start(out=slb_sb[:], in_=slb_d.ap()[:, :])
            nc.sync.dma_start(out=iot_sb[:], in_=iot_d.ap()[:, :, :])
            nc.sync.dma_start(out=rcp_sb[:], in_=rcp_d.ap()[:, :])
            nc.sync.dma_start(out=wt_sb[:], in_=wt_d.ap()[:, :])
            nc.sync.dma_start(out=bia_sb[:], in_=bia_d.ap()[:, :])

            pooled_ps = psum_p.tile([128, BPC], f32)
            nc.vector.memset(pooled_ps[:], 0.0)

            s_tiles = {}

            def emit_chunk(ci):
                i0, i1, wc = chunks[ci]
                if wc == 0:
                    return
                ntk = i1 - i0
                s = s_p.tile([128, K_USED, WMAX], bf16, tag="s")
                nc.vector.tensor_tensor(
                    out=s[:, :ntk, :wc],
                    in0=slb_sb[:, i0:i1, None].to_broadcast([128, ntk, wc]),
                    in1=iot_sb[:, :ntk, :wc],
                    op=mybir.AluOpType.is_equal,
                )
                s_tiles[ci] = s

            # gathers (one per window) + S chunks interleaved in chunk order
            g_tiles = {}
            qrr = 0
            for (w, tw0, pw) in plan["gathers"]:
                g = g_p.tile([128, NT_W, 128], bf16, tag="g")
                nc.gpsimd.dma_gather(
                    out_ap=g[:, :pw // 128, :],
                    in_ap=emb_d.ap()[w * WIN:(w + 1) * WIN, :],
                    idxs_ap=idx_sb[:, (tw0 * 128) // 16:(tw0 * 128 + pw) // 16],
                    num_idxs=pw,
                    num_idxs_reg=pw,
                    elem_size=D,
                    queue_num=qrr % NQUEUES,
                    # single-packet coalescing caps a packet at 64 descriptors
                    # per engine; our per-window gathers emit up to ~273, so
                    # each descriptor must be its own packet.
                    single_packet=False,
                )
                qrr += 1
                g_tiles[w] = (g, tw0)
                for ci in chunks_after_gather.get(w, []):
                    emit_chunk(ci)

            # pooling matmuls, in tile order, sliding PSUM windows
            for (t, c0, s0, width) in segs:
                wnd = int(window_of_tile[t])
                g, tw0 = g_tiles[wnd]
                ci = int(chunk_of_tile[t])
                s = s_tiles[ci]
                nc.tensor.matmul(
                    out=pooled_ps[:, c0:c0 + width],
                    lhsT=g[:, t - tw0, :],
                    rhs=s[:, t - chunks[ci][0], s0:s0 + width],
                    start=False,
                    stop=False,
                    skip_group_check=True,
                )

            pooled_sb = res_p.tile([128, BPC], f32)
            nc.vector.tensor_tensor(
                out=pooled_sb[:],
                in0=pooled_ps[:],
                in1=rcp_sb[:],
                op=mybir.AluOpType.mult,
            )
            out_sb = res_p.tile([OUT_DIM, BPC], f32)
            for k in range(BPC // 512):
                lin_ps = psuml_p.tile([128, 512], f32, tag="lin")
                nc.tensor.matmul(
                    out=lin_ps[:OUT_DIM, :],
                    lhsT=wt_sb[:],
                    rhs=pooled_sb[:, k * 512:(k + 1) * 512],
                    start=True,
                    stop=True,
                )
                nc.vector.tensor_tensor(
                    out=out_sb[:, k * 512:(k + 1) * 512],
                    in0=lin_ps[:OUT_DIM, :],
                    in1=bia_sb[:OUT_DIM, 0:1].to_broadcast([OUT_DIM, 512]),
                    op=mybir.AluOpType.add,
                )
            nc.sync.dma_start(out=out_d.ap()[:, :], in_=out_sb[:])

    nc.finalize()
    return nc


def make_in_maps(plan, emb_table, lin_w, lin_b):
    emb_bf = np.ascontiguousarray(
        np.asarray(emb_table, dtype=np.float32).astype(ml_dtypes.bfloat16)
    )
    lin_w = np.asarray(lin_w, dtype=np.float32)
    lin_b = np.asarray(lin_b, dtype=np.float32)
    iota = np.ascontiguousarray(
        np.broadcast_to(
            np.arange(plan["WMAX"], dtype=np.float32).astype(ml_dtypes.bfloat16),
            (128, plan["K_USED"], plan["WMAX"]),
        )
    )
    wt = np.ascontiguousarray(lin_w.T)
    bia = np.zeros((128, 1), dtype=np.float32)
    bia[:OUT_DIM, 0] = lin_b
    in_maps = []
    for c in range(NCORES):
        pc = plan["cores"][c]
        in_maps.append({
            "emb": emb_bf,
            "idx": pc["idx16"],
            "slb": pc["slotb"],
            "iot": iota,
            "rcp": pc["recipb"],
            "wt": wt,
            "bia": bia,
        })
    return in_maps


def assemble_output(results):
    out = np.zeros((BATCH, OUT_DIM), dtype=np.float32)
    for c in range(NCORES):
        out[c * BPC:(c + 1) * BPC] = results[c]["out"].T
    return out


def kernel(emb_table, lin_w, lin_b, sparse_features, offsets, send_shape,
           trace=False):
    from concourse.bass_utils import run_bass_kernel_spmd

    plan = build_plan(sparse_features, offsets)
    nc = build_program(plan)
    in_maps = make_in_maps(plan, emb_table, lin_w, lin_b)
    res = run_bass_kernel_spmd(nc, in_maps, list(range(NCORES)), trace=trace)
    out = assemble_output(res.results)
    if trace:
        return out, res
    return out
